# revision 2
# baseline (speedup 1.0000x reference)
"""Paged-KV-cache causal GQA attention on 8 TRN2 NeuronCores.

Problem shape (hardcoded): B=8 seqs x S=1024 tokens, H=32 q-heads,
KVH=8 kv-heads (GQA group 4), D=128, block_size=256, 40 cache blocks.

Sharding: data parallel, one sequence per core. Host does the
store_kvcache scatter + block-table gather (layout work) and per-core
layout prep (head-major transposes + bf16 cast); each core runs causal
flash attention for its sequence over all 32 heads.

Device algorithm per (head, q-chunk of 512):
  scores^T[k,q] = (K^T tile).T @ Q^T chunk   (PE, bf16, f32 psum)
  P = exp(scores * scale)                    (ACT, psum->sbuf bf16, no
                                              max subtraction: scores ~N(0,1))
  diagonal 128x128 block masked by upper-tri 0/1 mult (DVE)
  O[q, 0:128] + rowsum[q] = P.T @ [V | 1]    (PE accumulate over k tiles)
  out = O * (1/rowsum)                       (DVE, psum->sbuf f32)
"""

import sys

import numpy as np
import ml_dtypes

sys.path.insert(0, "/opt/trn_rl_repo")

import concourse.bass as bass  # noqa: E402
import concourse.mybir as mybir  # noqa: E402
import concourse.tile as tile  # noqa: E402
from concourse import bacc  # noqa: E402
from concourse.bass_utils import run_bass_kernel_spmd  # noqa: E402

B, S = 8, 1024
H, KVH, D = 32, 8, 128
G = H // KVH
NT = S // 128  # 8 k/q tiles of 128 per sequence
VW = 132  # v tile row: 128 v cols + ones col + pad
SCALE = 1.0 / float(np.sqrt(D))
BF = mybir.dt.bfloat16
F32 = mybir.dt.float32
_NC = None


def _build_nc():
    nc = bacc.Bacc("TRN2", target_bir_lowering=False, debug=False, num_devices=8)
    qT = nc.dram_tensor("qT", [H, D, S], BF, kind="ExternalInput").ap()
    kT = nc.dram_tensor("kT", [KVH, D, S], BF, kind="ExternalInput").ap()
    v1 = nc.dram_tensor("v1", [KVH, NT, 128, VW], BF, kind="ExternalInput").ap()
    out = nc.dram_tensor("out", [H, S, D], F32, kind="ExternalOutput").ap()
    mask_np = np.triu(np.ones((128, 128), dtype=ml_dtypes.bfloat16))
    mask_dram = nc.inline_tensor(mask_np, "tri_mask").ap()

    with tile.TileContext(nc) as tc:
        with (
            tc.tile_pool(name="singles", bufs=1) as singles,
            tc.tile_pool(name="qpool", bufs=3) as qpool,
            tc.tile_pool(name="ppool", bufs=4) as ppool,
            tc.tile_pool(name="opool", bufs=4) as opool,
            tc.tile_pool(name="rpool", bufs=4) as rpool,
            tc.tile_pool(name="pspool", bufs=3, space="PSUM") as pspool,
            tc.tile_pool(name="popool", bufs=4, space="PSUM") as popool,
        ):
            mask_sb = singles.tile([128, 128], BF)
            nc.sync.dma_start(out=mask_sb, in_=mask_dram)
            kT_sb = singles.tile([128, KVH * S], BF)
            nc.sync.dma_start(
                out=kT_sb.rearrange("p (k s) -> p k s", k=KVH),
                in_=kT.rearrange("k d s -> d k s"),
            )
            v1_sb = singles.tile([128, KVH * NT * VW], BF)
            nc.sync.dma_start(
                out=v1_sb.rearrange("p (k t c) -> p k t c", k=KVH, t=NT),
                in_=v1.rearrange("k t p c -> p k t c"),
            )

            for h in range(H):
                kvh = h // G
                q_sb = qpool.tile([128, S], BF, tag="q")
                nc.sync.dma_start(out=q_sb, in_=qT[h])
                for qc in range(2):
                    po = [
                        popool.tile([128, 129], F32, tag="po", name=f"po_{h}_{qc}_{j}")
                        for j in range(4)
                    ]
                    for kt in range(qc * 4 + 4):
                        q_off = max(0, kt - qc * 4)
                        w = 512 - q_off * 128
                        ps = pspool.tile([128, w], F32, tag="ps")
                        nc.tensor.matmul(
                            ps,
                            lhsT=kT_sb[:, kvh * S + kt * 128 : kvh * S + kt * 128 + 128],
                            rhs=q_sb[:, qc * 512 + q_off * 128 : qc * 512 + 512],
                            start=True,
                            stop=True,
                        )
                        p_sb = ppool.tile([128, w], BF, tag="p")
                        # P = exp(scores * scale); safe without max subtraction
                        nc.scalar.activation(
                            p_sb, ps, mybir.ActivationFunctionType.Exp, scale=SCALE
                        )
                        if kt >= qc * 4:
                            nc.vector.tensor_mul(
                                p_sb[:, 0:128], p_sb[:, 0:128], mask_sb
                            )
                        for j in range(q_off, 4):
                            qt = qc * 4 + j
                            o = (j - q_off) * 128
                            nc.tensor.matmul(
                                po[j],
                                lhsT=p_sb[:, o : o + 128],
                                rhs=v1_sb[
                                    :, (kvh * NT + kt) * VW : (kvh * NT + kt) * VW + 129
                                ],
                                start=(kt == 0),
                                stop=(kt == qt),
                                skip_group_check=True,
                            )
                    for j in range(4):
                        qt = qc * 4 + j
                        recip = rpool.tile([128, 1], F32, tag="r")
                        nc.vector.reciprocal(recip, po[j][:, 128:129])
                        osb = opool.tile([128, 128], F32, tag="o")
                        nc.vector.tensor_scalar_mul(osb, po[j][:, 0:128], recip)
                        nc.sync.dma_start(
                            out=out[h, qt * 128 : (qt + 1) * 128, :], in_=osb
                        )

    nc.compile()
    return nc


def _get_nc():
    global _NC
    if _NC is None:
        _NC = _build_nc()
    return _NC


def kernel(q, k, v, k_cache, v_cache, slot_mapping, block_tables):
    out_dtype = q.dtype
    nb, bs, kvh, d = k_cache.shape
    # store_kvcache scatter (mirrors reference semantics on host)
    kc = k_cache.reshape(nb * bs, kvh, d).copy()
    vc = v_cache.reshape(nb * bs, kvh, d).copy()
    kc[slot_mapping] = k
    vc[slot_mapping] = v
    b, mb = block_tables.shape
    s = q.shape[0] // b
    pos = np.arange(s)
    slot_grid = block_tables[:, pos // bs] * bs + (pos % bs)  # [B, S]
    kf = kc[slot_grid]  # [B, S, KVH, D]
    vf = vc[slot_grid]
    qb = q.reshape(b, s, H, D)

    bf16 = ml_dtypes.bfloat16
    in_maps = []
    for i in range(b):
        qTi = np.ascontiguousarray(qb[i].transpose(1, 2, 0)).astype(bf16)
        kTi = np.ascontiguousarray(kf[i].transpose(1, 2, 0)).astype(bf16)
        vh = vf[i].transpose(1, 0, 2).reshape(KVH, NT, 128, D)
        v1i = np.zeros((KVH, NT, 128, VW), dtype=bf16)
        v1i[..., :D] = vh.astype(bf16)
        v1i[..., D] = 1.0
        in_maps.append({"qT": qTi, "kT": kTi, "v1": v1i})

    nc = _get_nc()
    res = run_bass_kernel_spmd(nc, in_maps, core_ids=list(range(8)))
    outs = [res.results[i]["out"].transpose(1, 0, 2) for i in range(b)]  # [S, H, D]
    return np.concatenate(outs, axis=0).astype(out_dtype, copy=False)


if __name__ == "__main__":
    rng = np.random.default_rng(0)
    pass
